# revision 3
# baseline (speedup 1.0000x reference)
"""v6 (v5 + gpsimd x-copy): LSTM (B=1024, T=2048, D=1, H=50) + final Dense, on 8 TRN2 NeuronCores.

v3: data parallel 8 x 128; each core splits its 128 rows into TWO
independent 64-row chains (A: 0..63, B: 64..127) interleaved to hide the
per-step recurrence latency (~2us) behind engine throughput (~1.5us/round).

Per-chain state tile hcat [97, 64] bf16:
  rows 0..49 = h, row 64 = ones (bias), row 96 = x_t  (rows 64/96 chosen so
  their writes start at a legal partition base; pad rows are zero).
Stationary weights [97, 128]: Wh at rows 0..49, b at 64, Wx at 96; columns
bank-if = [f: 0..49 | i: 64..113], bank-go = [o: 0..49 | 2*g: 64..113].

Per chain-step: one PSUM tile z [128, 128] (cols 0..63 = bank-if, 64..127 =
bank-go) filled by TWO matmuls (start/stop closed per 64-col slice), then
ONE sigmoid over the whole tile -> g [128,128] bf16 (the g-gate tanh uses
sigmoid(2x) with host-side 2x column scaling), then on DVE:
  tt = f*c (f32), mh = (sg-.5)*i (bf16), c = 2*mh + tt (f32),
  h = o*tanh(c) (tanh on ACT, bf16), plus a 1-row copy of x_{t+1}.
c stays fp32 for accuracy; gates/h are bf16 for DVE 2x mode.
"""

import os
from contextlib import ExitStack

import numpy as np
import ml_dtypes

import concourse.bass as bass
import concourse.bacc as bacc
import concourse.mybir as mybir
import concourse.tile as tile
from concourse import bass_utils

B_TOTAL = 1024
N_CORES = 8
B = B_TOTAL // N_CORES  # 128 per core
B2 = B // 2  # 64 per chain
H = 50
K = 97  # hcat rows: h 0..49, ones 64, x 96
RCH = 16  # steps per ring DMA chunk

F32 = mybir.dt.float32
BF16 = mybir.dt.bfloat16

_CACHE = {}


def _build(t_steps: int):
    nc = bacc.Bacc()

    assert t_steps % RCH == 0
    n_ring = t_steps // RCH

    wif_d = nc.dram_tensor("w_if", [K, 128], BF16, kind="ExternalInput")
    wgo_d = nc.dram_tensor("w_go", [K, 128], BF16, kind="ExternalInput")
    # final-dense stationary rows: Wd at 0..49, bd at 64
    wdbd_d = nc.dram_tensor("wd_bd", [65, 1], F32, kind="ExternalInput")
    # ring: row j = x for steps [j*16, j*16+16), step-major x batch-minor
    xrA_d = nc.dram_tensor("xrA", [n_ring, RCH * B2], BF16, kind="ExternalInput")
    xrB_d = nc.dram_tensor("xrB", [n_ring, RCH * B2], BF16, kind="ExternalInput")
    y_d = nc.dram_tensor("y", [B, 1], F32, kind="ExternalOutput")

    Sig = mybir.ActivationFunctionType.Sigmoid
    Tanh = mybir.ActivationFunctionType.Tanh
    Op = mybir.AluOpType

    with tile.TileContext(nc) as tc, ExitStack() as ctx:
        cpool = ctx.enter_context(tc.tile_pool(name="const", bufs=1))
        spool = ctx.enter_context(tc.tile_pool(name="state", bufs=1))
        gApool = ctx.enter_context(tc.tile_pool(name="gA", bufs=2))
        gBpool = ctx.enter_context(tc.tile_pool(name="gB", bufs=2))
        dApool = ctx.enter_context(tc.tile_pool(name="dA", bufs=2))
        dBpool = ctx.enter_context(tc.tile_pool(name="dB", bufs=2))
        rApool = ctx.enter_context(tc.tile_pool(name="rA", bufs=3))
        rBpool = ctx.enter_context(tc.tile_pool(name="rB", bufs=3))
        zApool = ctx.enter_context(tc.tile_pool(name="zA", bufs=2, space="PSUM"))
        zBpool = ctx.enter_context(tc.tile_pool(name="zB", bufs=2, space="PSUM"))
        ypool = ctx.enter_context(tc.tile_pool(name="yps", bufs=1, space="PSUM"))

        def load_const(name, dram, shape, dt):
            stg = cpool.tile(shape, dt, name=name + "_s", tag=name + "_s")
            nc.sync.dma_start(stg[:], dram[:])
            fin = cpool.tile(shape, dt, name=name, tag=name)
            nc.vector.tensor_copy(fin[:], stg[:])
            return fin

        wif = load_const("wif", wif_d, [K, 128], BF16)
        wgo = load_const("wgo", wgo_d, [K, 128], BF16)
        wdbd = load_const("wdbd", wdbd_d, [65, 1], F32)

        # --- state ---
        hcat = {}
        cst = {}
        for X in ("A", "B"):
            hcat[X] = spool.tile([K, B2], BF16, name="hcat" + X, tag="hcat" + X)
            nc.vector.memset(hcat[X][:], 0.0)
            nc.vector.memset(hcat[X][64:65, :], 1.0)
            cst[X] = spool.tile([H, B2], F32, name="cst" + X, tag="cst" + X)
            nc.vector.memset(cst[X][:], 0.0)
        hcatf = spool.tile([65, B], F32, name="hcatf", tag="hcatf")
        nc.vector.memset(hcatf[:], 0.0)
        nc.vector.memset(hcatf[64:65, :], 1.0)

        # --- x ring ---
        ring = {"A": {}, "B": {}}
        ring_dram = {"A": xrA_d, "B": xrB_d}
        ring_pool = {"A": rApool, "B": rBpool}

        def fetch_ring(X, j):
            if j * RCH < t_steps and j not in ring[X]:
                rt = ring_pool[X].tile(
                    [1, RCH * B2], BF16, name="ring" + X, tag="ring" + X
                )
                nc.gpsimd.dma_start(rt[:], ring_dram[X][j : j + 1, :])
                ring[X][j] = rt

        def ring_slot(X, t):
            rt = ring[X][t // RCH]
            s = t % RCH
            return rt[0:1, s * B2 : (s + 1) * B2]

        for X in ("A", "B"):
            fetch_ring(X, 0)
            fetch_ring(X, 1)
            # x_0 into hcat row 96
            nc.vector.tensor_copy(hcat[X][96:97, :], ring_slot(X, 0))

        for t in range(t_steps):
            if t % RCH == 0:
                j = t // RCH
                for X in ("A", "B"):
                    fetch_ring(X, j + 2)
                    ring[X].pop(j - 1, None)

            for X in ("A", "B"):
                zpool = zApool if X == "A" else zBpool
                gpool = gApool if X == "A" else gBpool
                dpool = dApool if X == "A" else dBpool

                zt = zpool.tile([128, 2 * B2], F32, name="z" + X, tag="z" + X)
                nc.tensor.matmul(
                    zt[:, 0:B2], wif[:], hcat[X][:], start=True, stop=True
                )
                nc.tensor.matmul(
                    zt[:, B2 : 2 * B2], wgo[:], hcat[X][:], start=True, stop=True
                )
                if t + 1 < t_steps:
                    # refresh x row for the next step while ACT works on this
                    # (on GPSIMD: the DVE is the bottleneck engine)
                    nc.gpsimd.tensor_copy(
                        hcat[X][96:97, :], ring_slot(X, t + 1)
                    )

                g = gpool.tile([128, 2 * B2], F32, name="g" + X, tag="g" + X)
                nc.scalar.activation(g[:], zt[:], Sig)

                ff = g[0:H, 0:B2]
                ii = g[64 : 64 + H, 0:B2]
                oo = g[0:H, B2 : 2 * B2]
                sg = g[64 : 64 + H, B2 : 2 * B2]

                tt = dpool.tile([H, B2], F32, name="tt" + X, tag="tt" + X)
                nc.vector.tensor_mul(tt[:], ff, cst[X][:])  # f * c_old
                mh = dpool.tile([H, B2], F32, name="mh" + X, tag="mh" + X)
                # (sg - 0.5) * i  ==  i * g / 2
                nc.vector.scalar_tensor_tensor(
                    mh[:], sg, 0.5, ii, Op.subtract, Op.mult
                )
                # c = 2*mh + tt = i*g + f*c
                nc.vector.scalar_tensor_tensor(
                    cst[X][:], mh[:], 2.0, tt[:], Op.mult, Op.add
                )
                tch = dpool.tile([H, B2], BF16, name="tch" + X, tag="tch" + X)
                nc.scalar.activation(tch[:], cst[X][:], Tanh)
                if t < t_steps - 1:
                    nc.vector.tensor_mul(hcat[X][0:H, :], oo, tch[:])
                else:
                    xoff = 0 if X == "A" else B2
                    nc.vector.tensor_mul(
                        hcatf[0:H, xoff : xoff + B2], oo, tch[:]
                    )

        yps = ypool.tile([B, 1], F32)
        nc.tensor.matmul(yps[:], hcatf[:], wdbd[:], start=True, stop=True)
        ysb = cpool.tile([B, 1], F32, name="ysb", tag="ysb")
        nc.vector.tensor_copy(ysb[:], yps[:])
        nc.sync.dma_start(y_d[:], ysb[:])

    nc.compile()
    return nc


def _prep_weights(Wx, Wh, b, Wd, bd):
    Wx = np.asarray(Wx, np.float32)
    Wh = np.asarray(Wh, np.float32)
    b = np.asarray(b, np.float32)
    Wd = np.asarray(Wd, np.float32)
    bd = np.asarray(bd, np.float32)

    # reference gate column order: i, f, g, o (50 each)
    i_sl, f_sl, g_sl, o_sl = (slice(k * H, (k + 1) * H) for k in range(4))

    def pack(colsA, colsB, scaleB=1.0):
        w = np.zeros((K, 128), np.float32)
        for cols, base, scale in ((colsA, 0, 1.0), (colsB, 64, scaleB)):
            w[0:H, base : base + H] = scale * Wh[:, cols]
            w[64, base : base + H] = scale * b[cols]
            w[96, base : base + H] = scale * Wx[0, cols]
        return w

    w_if = pack(f_sl, i_sl)
    w_go = pack(o_sl, g_sl, scaleB=2.0)

    wd_bd = np.zeros((65, 1), np.float32)
    wd_bd[0:H, 0] = Wd[:, 0]
    wd_bd[64, 0] = bd[0]

    bf = ml_dtypes.bfloat16
    return {
        "w_if": w_if.astype(bf),
        "w_go": w_go.astype(bf),
        "wd_bd": wd_bd,
    }


def _pack_ring(x2chain, t_steps):
    # x2chain: [B2, T] fp32 -> [T/RCH, RCH*B2] bf16, step-major batch-minor
    n_ring = t_steps // RCH
    out = np.empty((n_ring, RCH * B2), np.float32)
    for j in range(n_ring):
        out[j] = x2chain[:, j * RCH : (j + 1) * RCH].T.reshape(-1)
    return out.astype(ml_dtypes.bfloat16)


LAST_RESULTS = None


def kernel(inputs, Wx, Wh, b, Wd, bd):
    global LAST_RESULTS
    x = np.asarray(inputs, np.float32)
    Bt, t_steps, D = x.shape
    assert D == 1
    x2 = x[:, :, 0]

    key = t_steps
    if key not in _CACHE:
        _CACHE[key] = _build(t_steps)
    nc = _CACHE[key]

    w = _prep_weights(Wx, Wh, b, Wd, bd)

    n_cores = N_CORES
    bs = Bt // n_cores
    in_maps = []
    for c in range(n_cores):
        m = dict(w)
        xs = x2[c * bs : (c + 1) * bs, :]
        m["xrA"] = _pack_ring(xs[0:B2], t_steps)
        m["xrB"] = _pack_ring(xs[B2:B], t_steps)
        in_maps.append(m)

    trace = bool(int(os.environ.get("LSTM_TRACE", "0")))
    res = bass_utils.run_bass_kernel_spmd(
        nc, in_maps, core_ids=list(range(n_cores)), trace=trace
    )
    LAST_RESULTS = res
    y = np.concatenate([r["y"] for r in res.results], axis=0)
    return y.astype(np.float32)
